# revision 1
# baseline (speedup 1.0000x reference)
"""MoE expert-parallel FFN kernel for Trainium2 (8 NeuronCores).

Problem: x [4, 16384, 1024]; 8 experts, expert e applies
    y = gelu(x_chunk @ w1[e] + b1[e]) @ w2[e] + b2[e]
to tokens [e*2048:(e+1)*2048] of every group (chunk along dim 1).

Sharding: expert-parallel, one expert per core. Each core runs an
identical program on its own x chunk (8192 tokens) and expert weights.
No collectives.

Per-core kernel layout (all matmuls in f32r = full-rate fp32):
  host passes xT = x_chunk.T  [D, T] so both matmuls need no on-device
  transposes:
    mm1: hT[f,t]  = w1[d,f].T @ xT[d,t]   (lhsT = w1 tile, rhs = xT tile)
    mm2: yT[d,t]  = w2[f,d].T @ hT[f,t]   (lhsT = w2 tile, rhs = hT tile)
  d_ff (4096) is split in two resident phases of 2048 (w1+w2 halves =
  16 MB SBUF); each phase streams all tokens; phase 0 writes partial
  yT to a DRAM scratch, phase 1 adds its contribution and writes yT.
"""

import os
import sys

import numpy as np

for _p in ("/opt/trn_rl_repo", "/root/.axon_site/_ro/trn_rl_repo"):
    if os.path.isdir(_p) and _p not in sys.path:
        sys.path.insert(0, _p)

import concourse.bass as bass  # noqa: E402
import concourse.tile as tile  # noqa: E402
from concourse import bacc, mybir  # noqa: E402
from concourse.bass_utils import run_bass_kernel_spmd  # noqa: E402

# Problem shape (hardcoded per contract)
E = 8          # experts == cores
G = 4          # groups
TFULL = 16384  # tokens per group
D = 1024       # d_model
F = 4096       # d_ff
C = TFULL // E     # tokens per expert chunk per group (2048)
T = G * C          # tokens per core (8192)

TB = 512           # token block (matmul free dim)
NTB = T // TB      # 16
FBLK = 2048        # d_ff per phase
NPH = F // FBLK    # 2
KD = D // 128      # 8  k-tiles over d_model
MF = FBLK // 128   # 16 d_ff tiles per phase
MD = D // 128      # 8  d_model output tiles

f32 = mybir.dt.float32
f32r = mybir.dt.float32r

_NC_CACHE = {}


def _build_nc(repeats=1, psum_bufs=4, y_bufs=3, x_first=True, mmdt=f32r):
    nc = bacc.Bacc()
    xT = nc.dram_tensor("xT", [D, T], mmdt, kind="ExternalInput")
    w1 = nc.dram_tensor("w1", [D, F], mmdt, kind="ExternalInput")
    b1 = nc.dram_tensor("b1", [F], f32, kind="ExternalInput")
    w2 = nc.dram_tensor("w2", [F, D], mmdt, kind="ExternalInput")
    b2 = nc.dram_tensor("b2", [D], f32, kind="ExternalInput")
    yT = nc.dram_tensor("yT", [D, T], f32, kind="ExternalOutput")

    xTr = xT.rearrange("(k p) t -> p k t", p=128)    # [128, KD, T]
    w1r = w1.rearrange("(k p) f -> p k f", p=128)    # [128, KD, F]
    w2r = w2.rearrange("(m p) d -> p m d", p=128)    # [128, F//128, D]
    b1r = b1.rearrange("(m p) -> p m", p=128)        # [128, F//128]
    b2r = b2.rearrange("(m p) -> p m", p=128)        # [128, MD]

    gelu = mybir.ActivationFunctionType.Gelu

    with tile.TileContext(nc) as tc:
        with tc.tile_pool(name="wpool", bufs=1) as wpool, \
             tc.tile_pool(name="xpool", bufs=2) as xpool, \
             tc.tile_pool(name="hpool", bufs=1) as hpool, \
             tc.tile_pool(name="ypool", bufs=y_bufs) as ypool, \
             tc.tile_pool(name="bpool", bufs=1) as bpool, \
             tc.tile_pool(name="dram", bufs=1, space="DRAM") as dpool, \
             tc.tile_pool(name="psum", bufs=psum_bufs, space="PSUM") as psum:

            y0 = dpool.tile([D, T], f32)
            b2t = bpool.tile([128, MD], f32)
            nc.sync.dma_start(b2t, b2r)

            for ph in [p for _ in range(repeats) for p in range(NPH)]:
                # split weight loads per k-tile: the first mm1 group only
                # needs w1t[:, 0] + xt, so the PE starts ~20-30us earlier
                # per phase than with one monolithic 8.4MB DMA
                w1t = wpool.tile([128, KD, FBLK], mmdt, tag="w1t")
                for k in range(KD):
                    nc.sync.dma_start(w1t[:, k, :],
                                      w1r[:, k, ph * FBLK:(ph + 1) * FBLK])
                b1t = bpool.tile([128, MF], f32, tag="b1t")
                nc.sync.dma_start(b1t, b1r[:, ph * MF:(ph + 1) * MF])
                xt0 = None
                if x_first:
                    # first token block's x before w2: mm1 can start sooner
                    xt0 = xpool.tile([128, KD, TB], mmdt, tag="xt")
                    nc.sync.dma_start(xt0, xTr[:, :, 0:TB])
                w2t = wpool.tile([128, MF, D], mmdt, tag="w2t")
                for m4 in range(0, MF, 4):
                    nc.sync.dma_start(w2t[:, m4:m4 + 4, :],
                                      w2r[:, ph * MF + m4:ph * MF + m4 + 4, :])

                for tb in range(NTB):
                    t0 = tb * TB
                    if tb == 0 and xt0 is not None:
                        xt = xt0
                    else:
                        xt = xpool.tile([128, KD, TB], mmdt, tag="xt")
                        nc.sync.dma_start(xt, xTr[:, :, t0:t0 + TB])

                    ht = hpool.tile([128, MF, TB], mmdt, tag="ht")
                    for m in range(MF):
                        ps = psum.tile([128, TB], f32, tag="ps1")
                        for k in range(KD):
                            nc.tensor.matmul(
                                ps,
                                lhsT=w1t[:, k, m * 128:(m + 1) * 128],
                                rhs=xt[:, k, :],
                                start=(k == 0),
                                stop=(k == KD - 1),
                            )
                        nc.scalar.activation(ht[:, m, :], ps, gelu,
                                             bias=b1t[:, m:m + 1])

                    for mo in range(MD):
                        ps2 = psum.tile([128, TB], f32, tag="ps2")
                        for m in range(MF):
                            nc.tensor.matmul(
                                ps2,
                                lhsT=w2t[:, m, mo * 128:(mo + 1) * 128],
                                rhs=ht[:, m, :],
                                start=(m == 0),
                                stop=(m == MF - 1),
                            )
                        if ph == 0:
                            yt = ypool.tile([128, TB], f32, tag="yt")
                            nc.vector.tensor_scalar_add(yt, ps2,
                                                        b2t[:, mo:mo + 1])
                            nc.sync.dma_start(
                                y0[mo * 128:(mo + 1) * 128, t0:t0 + TB], yt)
                        else:
                            y0t = ypool.tile([128, TB], f32, tag="y0t")
                            nc.sync.dma_start(
                                y0t, y0[mo * 128:(mo + 1) * 128, t0:t0 + TB])
                            yt = ypool.tile([128, TB], f32, tag="yt")
                            nc.vector.tensor_add(yt, ps2, y0t)
                            nc.sync.dma_start(
                                yT[mo * 128:(mo + 1) * 128, t0:t0 + TB], yt)

    nc.compile()
    return nc


def _get_nc():
    if "nc" not in _NC_CACHE:
        _NC_CACHE["nc"] = _build_nc()
    return _NC_CACHE["nc"]


def kernel(x, w1, b1, w2, b2, _trace=False, _trace_kwargs=None):
    x = np.asarray(x, dtype=np.float32)
    w1 = np.asarray(w1, dtype=np.float32)
    b1 = np.asarray(b1, dtype=np.float32)
    w2 = np.asarray(w2, dtype=np.float32)
    b2 = np.asarray(b2, dtype=np.float32)

    nc = _get_nc()
    xe = x.reshape(G, E, C, D)
    in_maps = []
    for e in range(E):
        xc = np.ascontiguousarray(xe[:, e].reshape(T, D).T)  # [D, T]
        in_maps.append({
            "xT": xc,
            "w1": np.ascontiguousarray(w1[e]),
            "b1": np.ascontiguousarray(b1[e]),
            "w2": np.ascontiguousarray(w2[e]),
            "b2": np.ascontiguousarray(b2[e]),
        })

    kw = dict(_trace_kwargs or {})
    try:
        res = run_bass_kernel_spmd(nc, in_maps, list(range(E)),
                                   trace=_trace, **kw)
    except Exception:
        # transient device wedge (e.g. NRT_EXEC_UNIT_UNRECOVERABLE) — retry
        res = run_bass_kernel_spmd(nc, in_maps, list(range(E)),
                                   trace=_trace, **kw)

    out = np.empty((G, TFULL, D), dtype=np.float32)
    for e in range(E):
        yTv = res.results[e]["yT"]                    # [D, T]
        out[:, e * C:(e + 1) * C, :] = yTv.T.reshape(G, C, D)

    if _trace:
        kernel.last_exec_time_ns = res.exec_time_ns
        kernel.last_results = res
    return out



# revision 4
# speedup vs baseline: 1.2983x; 1.2983x over previous
"""MoE expert-parallel FFN kernel for Trainium2 (8 NeuronCores).

Problem: x [4, 16384, 1024]; 8 experts, expert e applies
    y = gelu(x_chunk @ w1[e] + b1[e]) @ w2[e] + b2[e]
to tokens [e*2048:(e+1)*2048] of every group (chunk along dim 1).

Sharding: expert-parallel, one expert per core; each core runs an
identical program on its own x chunk (8192 tokens) and expert weights.

Per-core math: split-precision fp8 with DoubleRow matmuls. Every
operand is pre-split (host side) into a pair of fp8e4 tensors
(hi + residual) at power-of-2 scales:
    16*x   = xa + xb,   256*w = wa + wb   (elementwise, both fp8)
Each 256-deep K-chunk of a GEMM is computed with three DoubleRow
matmuls accumulating into the same PSUM tile:
    xa@wa + xb@wa + xa@wb        (the xb@wb term is ~1e-3 and dropped)
which recovers near-bf16 accuracy while DoubleRow contracts 256
elements per instruction. gelu runs on the scalar engine reading PSUM
(descale 1/4096 fused into the activation scale); the DVE re-splits
h into fp8 pairs (scale 32) for the second GEMM; the final output is
descaled by 1/8192 and biased in a single DVE op.
"""

import os
import sys

import numpy as np

for _p in ("/opt/trn_rl_repo", "/root/.axon_site/_ro/trn_rl_repo"):
    if os.path.isdir(_p) and _p not in sys.path:
        sys.path.insert(0, _p)

import ml_dtypes  # noqa: E402

import concourse.bass as bass  # noqa: E402
import concourse.tile as tile  # noqa: E402
from concourse import bacc, mybir  # noqa: E402
from concourse.bass_utils import run_bass_kernel_spmd  # noqa: E402

# Problem shape (hardcoded per contract)
E = 8          # experts == cores
G = 4          # groups
TFULL = 16384  # tokens per group
D = 1024       # d_model
F = 4096       # d_ff
C = TFULL // E     # tokens per expert chunk per group (2048)
T = G * C          # tokens per core (8192)

TB = 512           # token block (matmul free dim)
NTB = T // TB      # 16
KC = D // 256      # 4   256-deep K chunks over d_model
FC = F // 256      # 16  256-deep K chunks over d_ff
MF = F // 128      # 32  d_ff psum tiles (mm1 outputs)
MD = D // 128      # 8   d_model psum tiles (mm2 outputs)

SX = 16.0          # x scale
SW = 256.0         # w1/w2 scale
SH = 32.0          # h scale
S1INV = 1.0 / (SX * SW)   # psum1 descale (into gelu)
S2INV = 1.0 / (SH * SW)   # psum2 descale (into y)

F8NP = ml_dtypes.float8_e4m3

f32 = mybir.dt.float32
fp8 = mybir.dt.float8e4
DR = mybir.MatmulPerfMode.DoubleRow

_NC_CACHE = {}


def _build_nc():
    nc = bacc.Bacc()
    xa = nc.dram_tensor("xa", [128, KC, 2, T], fp8, kind="ExternalInput")
    xb = nc.dram_tensor("xb", [128, KC, 2, T], fp8, kind="ExternalInput")
    w1a = nc.dram_tensor("w1a", [128, KC, 2, F], fp8, kind="ExternalInput")
    w1b = nc.dram_tensor("w1b", [128, KC, 2, F], fp8, kind="ExternalInput")
    w2a = nc.dram_tensor("w2a", [128, FC, 2, D], fp8, kind="ExternalInput")
    w2b = nc.dram_tensor("w2b", [128, FC, 2, D], fp8, kind="ExternalInput")
    b1 = nc.dram_tensor("b1", [128, MF], f32, kind="ExternalInput")
    b2s = nc.dram_tensor("b2s", [128, MD], f32, kind="ExternalInput")
    yT = nc.dram_tensor("yT", [D, T], f32, kind="ExternalOutput")

    gelu = mybir.ActivationFunctionType.Gelu
    add = mybir.AluOpType.add
    mult = mybir.AluOpType.mult
    subtract = mybir.AluOpType.subtract

    with tile.TileContext(nc) as tc:
        with tc.tile_pool(name="wpool", bufs=1) as wpool, \
             tc.tile_pool(name="xpool", bufs=3) as xpool, \
             tc.tile_pool(name="hfpool", bufs=3) as hfpool, \
             tc.tile_pool(name="hapool", bufs=18) as hapool, \
             tc.tile_pool(name="hbpool", bufs=18) as hbpool, \
             tc.tile_pool(name="ypool", bufs=3) as ypool, \
             tc.tile_pool(name="ps1", bufs=3, space="PSUM") as ps1, \
             tc.tile_pool(name="ps2", bufs=2, space="PSUM") as ps2:

            b1t = wpool.tile([128, MF], f32, tag="b1t")
            nc.sync.dma_start(b1t, b1[:, :])
            b2t = wpool.tile([128, MD], f32, tag="b2t")
            nc.sync.dma_start(b2t, b2s[:, :])

            w1at = wpool.tile([128, KC, 2, F], fp8, tag="w1at")
            w1bt = wpool.tile([128, KC, 2, F], fp8, tag="w1bt")
            w2at = wpool.tile([128, FC, 2, D], fp8, tag="w2at")
            w2bt = wpool.tile([128, FC, 2, D], fp8, tag="w2bt")

            # first f-slice of w1 small so the PE can start ~immediately;
            # w2 needed only ~40us in
            for lo, hi in ((0, 512), (512, 2048), (2048, F)):
                nc.sync.dma_start(w1at[:, :, :, lo:hi], w1a[:, :, :, lo:hi])
                nc.sync.dma_start(w1bt[:, :, :, lo:hi], w1b[:, :, :, lo:hi])
            for lo, hi in ((0, 4), (4, 10), (10, FC)):
                nc.sync.dma_start(w2at[:, lo:hi], w2a[:, lo:hi])
                nc.sync.dma_start(w2bt[:, lo:hi], w2b[:, lo:hi])

            for tb in range(NTB):
                t0 = tb * TB
                xat = xpool.tile([128, KC, 2, TB], fp8, tag="xat")
                nc.sync.dma_start(xat, xa[:, :, :, t0:t0 + TB])
                xbt = xpool.tile([128, KC, 2, TB], fp8, tag="xbt")
                nc.sync.dma_start(xbt, xb[:, :, :, t0:t0 + TB])

                ha_c = []
                hb_c = []
                for m in range(MF):
                    ps = ps1.tile([128, TB], f32, tag="ps1")
                    last = 3 * KC - 1
                    idx = 0
                    for c in range(KC):
                        for lhsT, rhs in ((w1at, xat), (w1at, xbt),
                                          (w1bt, xat)):
                            nc.tensor.matmul(
                                ps,
                                lhsT=lhsT[:, c, :, m * 128:(m + 1) * 128],
                                rhs=rhs[:, c, :, :],
                                start=(idx == 0),
                                stop=(idx == last),
                                perf_mode=DR,
                            )
                            idx += 1
                    hf = hfpool.tile([128, TB], f32, tag="hf")
                    nc.scalar.activation(hf, ps, gelu,
                                         bias=b1t[:, m:m + 1], scale=S1INV)
                    c2, i2 = divmod(m, 2)
                    if i2 == 0:
                        ha_c.append(hapool.tile([128, 2, TB], fp8, tag="ha", name="ha"))
                        hb_c.append(hbpool.tile([128, 2, TB], fp8, tag="hb", name="hb"))
                    nc.vector.tensor_scalar_mul(ha_c[c2][:, i2, :], hf, SH)
                    nc.vector.scalar_tensor_tensor(
                        hb_c[c2][:, i2, :], hf, SH, ha_c[c2][:, i2, :],
                        op0=mult, op1=subtract)

                for mo in range(MD):
                    ps = ps2.tile([128, TB], f32, tag="ps2")
                    last = 3 * FC - 1
                    idx = 0
                    for c in range(FC):
                        for lhsT, rhs in ((w2at, ha_c[c]), (w2at, hb_c[c]),
                                          (w2bt, ha_c[c])):
                            nc.tensor.matmul(
                                ps,
                                lhsT=lhsT[:, c, :, mo * 128:(mo + 1) * 128],
                                rhs=rhs,
                                start=(idx == 0),
                                stop=(idx == last),
                                perf_mode=DR,
                            )
                            idx += 1
                    yt = ypool.tile([128, TB], f32, tag="yt")
                    nc.vector.tensor_scalar(yt, ps, b2t[:, mo:mo + 1],
                                            S2INV, op0=add, op1=mult)
                    nc.sync.dma_start(
                        yT[mo * 128:(mo + 1) * 128, t0:t0 + TB], yt)

    nc.compile()
    return nc


def _get_nc():
    if "nc" not in _NC_CACHE:
        _NC_CACHE["nc"] = _build_nc()
    return _NC_CACHE["nc"]


def _split_fp8(a, scale):
    """a*scale -> (hi, lo) fp8e4 pair with hi + lo ~= a*scale."""
    s = (a * scale).astype(np.float32)
    hi = s.astype(F8NP)
    lo = (s - hi.astype(np.float32)).astype(F8NP)
    return hi, lo


def _pack_k(a):
    """[K, N] with K = nc*256 -> [128, nc, 2, N] (k = c*256 + i*128 + p)."""
    k, n = a.shape
    return np.ascontiguousarray(
        a.reshape(k // 256, 2, 128, n).transpose(2, 0, 1, 3))


def kernel(x, w1, b1, w2, b2, _trace=False, _trace_kwargs=None):
    x = np.asarray(x, dtype=np.float32)
    w1 = np.asarray(w1, dtype=np.float32)
    b1 = np.asarray(b1, dtype=np.float32)
    w2 = np.asarray(w2, dtype=np.float32)
    b2 = np.asarray(b2, dtype=np.float32)

    nc = _get_nc()
    xe = x.reshape(G, E, C, D)
    in_maps = []
    for e in range(E):
        xc = xe[:, e].reshape(T, D).T  # [D, T]
        xa, xb = _split_fp8(xc, SX)
        w1a, w1b = _split_fp8(w1[e], SW)
        w2a, w2b = _split_fp8(w2[e], SW)
        in_maps.append({
            "xa": _pack_k(xa),
            "xb": _pack_k(xb),
            "w1a": _pack_k(w1a),
            "w1b": _pack_k(w1b),
            "w2a": _pack_k(w2a),
            "w2b": _pack_k(w2b),
            "b1": np.ascontiguousarray(b1[e].reshape(MF, 128).T),
            "b2s": np.ascontiguousarray(
                (b2[e] * SH * SW).reshape(MD, 128).T.astype(np.float32)),
        })

    kw = dict(_trace_kwargs or {})
    try:
        res = run_bass_kernel_spmd(nc, in_maps, list(range(E)),
                                   trace=_trace, **kw)
    except Exception:
        # transient device wedge (e.g. NRT_EXEC_UNIT_UNRECOVERABLE) — retry
        res = run_bass_kernel_spmd(nc, in_maps, list(range(E)),
                                   trace=_trace, **kw)

    out = np.empty((G, TFULL, D), dtype=np.float32)
    for e in range(E):
        yTv = res.results[e]["yT"]                    # [D, T]
        out[:, e * C:(e + 1) * C, :] = yTv.T.reshape(G, C, D)

    if _trace:
        kernel.last_exec_time_ns = res.exec_time_ns
        kernel.last_results = res
    return out
